# revision 15
# baseline (speedup 1.0000x reference)
"""Block-sparse linear y = x @ W^T + b on 8 TRN2 NeuronCores.

Problem shape (hardcoded): x [8192, 4096] f32, weight [1024, 64, 64] f32
(64x64 blocks), bias [4096] f32, row_idx/col_idx [1024] int32 over a 64x64
block grid.

Strategy: data-parallel over tokens (1024/core); y^T = W x^T + b with 64x64
block matmuls packed into PE-array quadrants (tile_position). Measured HW:
each quadrant sustains a 512-row bf16 matmul per ~216ns, 4 quadrants
concurrent (FLOP roofline ~110.6us/core); input DMA ~340GB/s shared across
3 queues (~2:1:1 gpsimd:sync:scalar arbitration).

Load-phase design: while x streams in, PSUM units are SINGLE-tile
"visits" — a (pair, token-half) accumulates one row-group (ki) at a time
into one PSUM bank, is partially evicted into its fp16 output tile
(scalar activation with bias on first touch of a region, vector add
later), and the bank moves to whichever unit has the most arrived work.
This keeps the tensor engine fed from 8 banks' worth of open units while
x trickles in. After x is resident, units open in dual-tile mode (2
banks, both row-groups concurrent — the baseline-proven layout). A
build-time discrete-event simulation produces the full static
instruction order. Cross-row-group PSUM write races are impossible by
construction: a single-tile visit only issues matmuls of one row-group.

The row-group (ki) of a block is its column parity (x^T tile t holds
columns 2t|2t+1 on partition halves); a few blocks are reassigned via
duplicated columns to balance the global even/odd load.
"""

from contextlib import ExitStack

import numpy as np
import ml_dtypes

import concourse.tile as tile
from concourse import bacc, mybir
from concourse.bass_utils import run_bass_kernel_spmd

BLK = 64
OUT_BLK = 64
IN_BLK = 64
D_IN = IN_BLK * BLK
D_OUT = OUT_BLK * BLK
N_CORES = 8
BF16 = ml_dtypes.bfloat16

MM_NS = 216.0
EVICT_GATE = 1500.0
T_BOOT = 7500.0
XQBW = 255.0           # gpsimd+sync combined, bytes/ns
WQBW = 85.0            # scalar queue, bytes/ns
N_BANKS = 8
WG_HEAD = 4            # leading weight groups are small for fast start
WG_HEAD_SLOTS = 8
WG_SLOTS = 32
SWAP_MIN = 6
MAX_SWAPS = 64


# ----------------------------------------------------------------- planning

def _dedupe(row_idx, col_idx):
    d = {}
    for i in range(len(row_idx)):
        d[(int(row_idx[i]), int(col_idx[i]))] = i
    blocks_by_r = [[] for _ in range(OUT_BLK)]
    for (r, c), w in d.items():
        blocks_by_r[r].append((c, w))
    for lst in blocks_by_r:
        lst.sort()
    return blocks_by_r


def _plan(row_idx, col_idx):
    blocks_by_r = _dedupe(row_idx, col_idx)
    ki_of = {}
    for r in range(OUT_BLK):
        for (c, w) in blocks_by_r[r]:
            ki_of[(r, c)] = c % 2
    total = sum(len(b) for b in blocks_by_r)
    n_even = sum(1 for v in ki_of.values() if v == 0)
    dup_cols = []
    excess = n_even - (total - n_even)
    if abs(excess) >= 2:
        m = abs(excess) // 2
        src_par = 0 if excess > 0 else 1
        usage = {}
        for (r, c) in ki_of:
            if c % 2 == src_par:
                usage[c] = usage.get(c, 0) + 1
        side = {r: sum(1 for (c, _) in blocks_by_r[r]
                       if ki_of[(r, c)] == src_par)
                for r in range(OUT_BLK)}
        for c in sorted(usage, key=lambda c: -usage[c]):
            if m <= 0:
                break
            dup_cols.append((c, 1 - src_par))
            users = sorted((r for r in range(OUT_BLK) if (r, c) in ki_of),
                           key=lambda r: -side[r])
            for r in users:
                if m <= 0:
                    break
                if ki_of[(r, c)] == src_par:
                    ki_of[(r, c)] = 1 - src_par
                    side[r] -= 1
                    m -= 1
    e_r = [sum(1 for (c, _) in blocks_by_r[r] if ki_of[(r, c)] == 0)
           for r in range(OUT_BLK)]
    order = sorted(range(OUT_BLK), key=lambda r: e_r[r])
    pairs = []
    for p in range(OUT_BLK // 2):
        a, b = order[p], order[OUT_BLK - 1 - p]
        pairs.append((a, b) if p % 2 == 0 else (b, a))
    return blocks_by_r, ki_of, dup_cols, pairs


def _x_resource(b):
    ki, mi, c, w = b
    return ("tile", c // 2) if (c % 2) == ki else ("dup", c)


def _wgroup_of_slot(idx):
    if idx < WG_HEAD * WG_HEAD_SLOTS:
        return idx // WG_HEAD_SLOTS, idx % WG_HEAD_SLOTS
    r = idx - WG_HEAD * WG_HEAD_SLOTS
    return WG_HEAD + r // WG_SLOTS, r % WG_SLOTS


def _wgroup_slots(g):
    return WG_HEAD_SLOTS if g < WG_HEAD else WG_SLOTS


# ---------------------------------------------------------------- schedule

def _schedule(blocks_by_r, ki_of, dup_cols, pairs, ntok):
    """Discrete-event schedule. Op list entries:
      ("alloc", ui, ntiles)
      ("mm", ui, b, start, stop, tix)
      ("evict", ui, regions, final)    regions: [(mi, tix, first), ...]
      ("store", p)
    """
    n_th = ntok // 512
    units, ublocks = [], []
    for p in range(len(pairs)):
        r1, r2 = pairs[p]
        blocks = []
        for mi, r in enumerate((r1, r2)):
            for (c, w) in blocks_by_r[r]:
                blocks.append((ki_of[(r, c)], mi, c, w))
        for th in range(n_th):
            units.append((p, th))
            ublocks.append(blocks)
    U = len(units)

    # weight slots, unit-sequential first-use order
    wslot = {}
    cnt = [0, 0]
    n_wgroups = 1
    for ui in range(U):
        for b in ublocks[ui]:
            if b[3] not in wslot:
                ki = b[0]
                idx = cnt[ki]
                cnt[ki] += 1
                g, j = _wgroup_of_slot(idx)
                wslot[b[3]] = (ki, g, j)
                n_wgroups = max(n_wgroups, g + 1)
    w_arrt = {}
    t = T_BOOT
    for g in range(n_wgroups):
        t += (_wgroup_slots(g) * BLK * BLK * 2 * 2) / WQBW
        w_arrt[g] = t

    # x tiles, first-use order
    x_order, seen = [], set()
    for ui in range(U):
        for b in ublocks[ui]:
            res = _x_resource(b)
            if res not in seen:
                seen.add(res)
                x_order.append(res)
    for tl in range(IN_BLK // 2):
        if ("tile", tl) not in seen:
            x_order.append(("tile", tl))
            seen.add(("tile", tl))
    x_arrt = {}
    t = T_BOOT
    for res in x_order:
        t += (128 if res[0] == "tile" else 64) * ntok * 2 / XQBW
        x_arrt[res] = t
    x_done_t = t

    def arr(b):
        return max(x_arrt[_x_resource(b)], w_arrt[wslot[b[3]][1]])

    qfree = {(ki, mi): 0.0 for ki in (0, 1) for mi in (0, 1)}
    remaining = [dict() for _ in range(U)]
    for ui, ub in enumerate(ublocks):
        for b in ub:
            remaining[ui].setdefault((b[0], b[1]), []).append(b)
    left = [len(ub) for ub in ublocks]
    ops = []
    open_u = {}
    unopened = list(range(U))
    touched = {}
    alloc_times = []       # per allocation index: evict-complete time
    unit_alloc = {}
    swaps = 0
    pair_units_left = [n_th] * len(pairs)

    def ready_single(ui, tl, ki):
        return sum(1 for mi in (0, 1)
                   for b in remaining[ui].get((ki, mi), [])
                   if arr(b) <= tl)

    def ready_all(ui, tl):
        return sum(1 for lst in remaining[ui].values()
                   for b in lst if arr(b) <= tl)

    def ring_gate_for(n_new):
        idx0 = len(alloc_times)
        g = 0.0
        for k in range(n_new):
            i = idx0 + k - N_BANKS
            if i >= 0:
                g = max(g, alloc_times[i])
        return g

    def alloc_unit(tl):
        if not unopened:
            return False
        dual = tl >= x_done_t
        nt = 2 if dual else 1
        in_use = sum(st["tiles"] for st in open_u.values())
        if in_use + nt > N_BANKS:
            return False
        gate = ring_gate_for(nt)
        if gate == float("inf"):
            return False
        unopened.sort(key=lambda u: (-ready_all(u, max(tl, gate)), u))
        ui = unopened.pop(0)
        ops.append(("alloc", ui, nt))
        open_u[ui] = {"mode": "dual" if dual else "single", "tiles": nt,
                      "gate": gate, "pend": {}, "lastki": {}}
        unit_alloc[ui] = list(range(len(alloc_times),
                                    len(alloc_times) + nt))
        alloc_times.extend([float("inf")] * nt)
        return True

    def close_unit(ui, tfin):
        st = open_u.pop(ui)
        regions = []
        for (mi, tix), n in sorted(st["pend"].items()):
            if n > 0:
                first = not touched.get((ui, mi), False)
                touched[(ui, mi)] = True
                regions.append((mi, tix, first))
        final = left[ui] == 0
        ops.append(("evict", ui, regions, final))
        for ai in unit_alloc.pop(ui):
            alloc_times[ai] = tfin + EVICT_GATE
        if final:
            p = units[ui][0]
            pair_units_left[p] -= 1
            if pair_units_left[p] == 0:
                ops.append(("store", p))
        else:
            unopened.append(ui)

    # per-quadrant emission counters for the region ki-switch rule:
    # a single-tile unit switching row-group on a region must have >=3
    # intervening matmuls on the target quadrant (in-order issue then
    # guarantees the previous row-group's writeback has fully landed)
    qcount = {(ki, mi): 0 for ki in (0, 1) for mi in (0, 1)}
    KSW = 3

    def switch_ok(st, ki, mi):
        lk = st["lastki"].get(mi)
        if lk is None or lk[0] == ki:
            return True
        return qcount[(ki, mi)] - lk[1][(ki, mi)] >= KSW

    while any(left):
        T_now = min(qfree.values())
        while unopened:
            if not alloc_unit(T_now):
                break
        best = None
        for q in sorted(qfree, key=lambda q: qfree[q]):
            ki, mi = q
            for ui, st in open_u.items():
                if st["mode"] == "single" and not switch_ok(st, ki, mi):
                    continue
                for b in remaining[ui].get(q, []):
                    t0 = max(qfree[q], st["gate"], arr(b))
                    key = (t0, left[ui], ui)
                    if best is None or key < best[0]:
                        best = (key, q, ui, b)
            if best is not None and best[0][0] <= qfree[q]:
                break
        if best is None:
            # all candidates blocked by switch rule: rotate a unit
            victim = max(open_u, key=lambda u: ready_all(u, T_now))
            close_unit(victim, T_now)
            continue
        key, q, ui, b = best
        stall = key[0] - T_now
        if stall > 400 and swaps < MAX_SWAPS and unopened:
            cands = [u for u in unopened
                     if ready_all(u, T_now) >= SWAP_MIN]
            if cands:
                victim = min(open_u, key=lambda u: ready_all(u, T_now))
                if ready_all(victim, T_now) < SWAP_MIN:
                    if left[victim] > 0:
                        swaps += 1
                    close_unit(victim, T_now)
                    continue
        st = open_u[ui]
        tix = b[0] if st["mode"] == "dual" else 0
        rkey = (b[1], tix)
        first = st["pend"].get(rkey, 0) == 0
        st["pend"][rkey] = st["pend"].get(rkey, 0) + 1
        ops.append(("mm", ui, b, first, False, tix))
        remaining[ui][q].remove(b)
        left[ui] -= 1
        qfree[q] = key[0] + MM_NS
        qcount[q] += 1
        if st["mode"] == "single":
            st["lastki"][b[1]] = (b[0], dict(qcount))
        if left[ui] == 0:
            close_unit(ui, key[0] + MM_NS)

    # bias-only regions never touched (rows with blocks on one side only)
    for ui in range(U):
        for mi in (0, 1):
            if ublocks[ui] and not touched.get((ui, mi), False):
                touched[(ui, mi)] = True   # handled in build via biasfill
    # post-pass: stop=True on last mm of each (visit, region)
    lastmm = {}
    visit_of = {}
    vcount = 0
    for i, op in enumerate(ops):
        if op[0] == "alloc":
            visit_of[op[1]] = vcount
            vcount += 1
        elif op[0] == "mm":
            lastmm[(visit_of[op[1]], op[2][1], op[5])] = i
    for i in set(lastmm.values()):
        op = ops[i]
        ops[i] = (op[0], op[1], op[2], op[3], True, op[5])
    makespan = max(qfree.values())
    return units, ops, wslot, n_wgroups, x_order, makespan, swaps


# ------------------------------------------------------------------- build

def _build(blocks_by_r, ki_of, dup_cols, pairs, units, ops, wslot,
           n_wgroups, x_order, ntok):
    n_th = ntok // 512
    sdt = mybir.dt.bfloat16
    f32 = mybir.dt.float32
    f16 = mybir.dt.float16
    n_pairs = len(pairs)
    dup_half = dict(dup_cols)

    nc = bacc.Bacc("TRN2", target_bir_lowering=False, debug=False)
    xt_d = nc.dram_tensor("xt", [D_IN, ntok], sdt, kind="ExternalInput").ap()
    wg_d = []
    for g in range(n_wgroups):
        wg_d.append(nc.dram_tensor(
            f"wg{g}", [128, _wgroup_slots(g) * BLK], sdt,
            kind="ExternalInput").ap())
    bias_d = nc.dram_tensor("bias_pk", [128, n_pairs], f32,
                            kind="ExternalInput").ap()
    yt_d = nc.dram_tensor("yt", [D_OUT, ntok], f16,
                          kind="ExternalOutput").ap()

    # count of blocks per (unit, mi) to find regions needing bias fill
    ucnt = []
    for p, (r1, r2) in enumerate(pairs):
        cnt = {0: 0, 1: 0}
        for mi, r in enumerate((r1, r2)):
            cnt[mi] = len(blocks_by_r[r])
        ucnt.append(cnt)

    with tile.TileContext(nc) as tc:
        with ExitStack() as ctx:
            xpool = ctx.enter_context(tc.tile_pool(name="xp", bufs=1))
            wpool = ctx.enter_context(tc.tile_pool(name="wp", bufs=1))
            pspool = ctx.enter_context(
                tc.tile_pool(name="ps", bufs=N_BANKS, space="PSUM"))
            opool = ctx.enter_context(tc.tile_pool(name="op", bufs=26))
            bpool = ctx.enter_context(tc.tile_pool(name="bp", bufs=1))

            bias_sb = bpool.tile([128, n_pairs], f32, tag="bias",
                                 name="bias_sb")
            nc.sync.dma_start(bias_sb[:], bias_d[:])

            # weight groups on the scalar queue, in order
            wg_tiles = []
            for g in range(n_wgroups):
                wt = wpool.tile([128, _wgroup_slots(g) * BLK], sdt,
                                tag=f"w{g}", name=f"w{g}")
                nc.scalar.dma_start(wt[:], wg_d[g][:])
                wg_tiles.append(wt)

            # x tiles alternating gpsimd / sync, first-use order
            xengines = [nc.gpsimd, nc.sync]
            xtiles = {}
            for i, res in enumerate(x_order):
                eng = xengines[i % 2]
                if res[0] == "tile":
                    tl = res[1]
                    xt = xpool.tile([128, ntok], sdt, tag=f"x{tl}",
                                    name=f"x{tl}")
                    eng.dma_start(xt[:], xt_d[128 * tl:128 * (tl + 1), :])
                else:
                    c = res[1]
                    half = dup_half[c]
                    xt = xpool.tile([128, ntok], sdt, tag=f"xd{c}",
                                    name=f"xd{c}")
                    eng.dma_start(xt[64 * half:64 * half + 64, :],
                                  xt_d[64 * c:64 * (c + 1), :])
                xtiles[res] = xt

            def x_ap(b, th):
                ki = b[0]
                t = xtiles[_x_resource(b)]
                return t[ki * 64:(ki + 1) * 64, th * 512:(th + 1) * 512]

            psum = {}
            osb_of = {}
            biasfilled = set()

            def get_osb(p):
                if p not in osb_of:
                    osb_of[p] = opool.tile([128, ntok], f16, tag="o",
                                           name=f"o{p}")
                return osb_of[p]

            for op in ops:
                if op[0] == "alloc":
                    _, ui, nt = op
                    psum[ui] = [pspool.tile([128, 512], f32, tag="ps",
                                            name=f"ps{ui}_{len(psum)}_{k}")
                                for k in range(nt)]
                elif op[0] == "mm":
                    _, ui, b, start, stop, tix = op
                    ki, mi, c, w = b
                    p, th = units[ui]
                    kis, g, j = wslot[w]
                    lhsT = wg_tiles[g][kis * 64:(kis + 1) * 64,
                                       j * BLK:(j + 1) * BLK]
                    nc.tensor.matmul(
                        psum[ui][tix][mi * 64:(mi + 1) * 64, :],
                        lhsT, x_ap(b, th),
                        start=start, stop=stop,
                        tile_position=(ki * 64, mi * 64),
                        skip_group_check=True,
                    )
                elif op[0] == "evict":
                    _, ui, regions, final = op
                    p, th = units[ui]
                    osb = get_osb(p)
                    osl = osb[:, th * 512:(th + 1) * 512]
                    pt = psum.pop(ui)
                    # merge (mi=0, mi=1) same-tix same-first into full tile
                    done = set()
                    bytix = {}
                    for (mi, tix, first) in regions:
                        bytix.setdefault((tix, first), []).append(mi)
                    for (tix, first), mis in sorted(bytix.items()):
                        if sorted(mis) == [0, 1]:
                            if first:
                                nc.scalar.activation(
                                    osl, pt[tix][:],
                                    mybir.ActivationFunctionType.Identity,
                                    bias=bias_sb[:, p:p + 1], scale=1.0)
                            else:
                                nc.vector.tensor_add(osl, osl, pt[tix][:])
                        else:
                            for mi in mis:
                                oh = osl[mi * 64:(mi + 1) * 64, :]
                                src = pt[tix][mi * 64:(mi + 1) * 64, :]
                                if first:
                                    nc.scalar.activation(
                                        oh, src,
                                        mybir.ActivationFunctionType
                                        .Identity,
                                        bias=bias_sb[mi * 64:(mi + 1) * 64,
                                                     p:p + 1],
                                        scale=1.0)
                                else:
                                    nc.vector.tensor_add(oh, oh, src)
                elif op[0] == "store":
                    p = op[1]
                    osb = get_osb(p)
                    # fill never-touched regions with bias
                    for mi in (0, 1):
                        if ucnt[p][mi] == 0:
                            for th in range(n_th):
                                oh = osb[mi * 64:(mi + 1) * 64,
                                         th * 512:(th + 1) * 512]
                                nc.vector.memset(oh, 0.0)
                                nc.vector.tensor_scalar_add(
                                    oh, oh,
                                    bias_sb[mi * 64:(mi + 1) * 64,
                                            p:p + 1])
                    nc.gpsimd.dma_start(yt_d[128 * p:128 * (p + 1), :],
                                        osb[:])

            # pairs with zero blocks entirely
            for p in range(n_pairs):
                if ucnt[p][0] == 0 and ucnt[p][1] == 0:
                    osb = get_osb(p)
                    for th in range(n_th):
                        osl = osb[:, th * 512:(th + 1) * 512]
                        nc.vector.memset(osl, 0.0)
                        nc.vector.tensor_scalar_add(
                            osl, osl, bias_sb[:, p:p + 1])
                    nc.gpsimd.dma_start(yt_d[128 * p:128 * (p + 1), :],
                                        osb[:])
    nc.compile()
    return nc


# ---------------------------------------------------------------- validate

def replay(inputs, core=0):
    """Numpy emulation of the scheduled op list for one core (debug aid)."""
    x = np.asarray(inputs["x"], dtype=np.float32)
    weight = np.asarray(inputs["weight"], dtype=np.float32)
    bias = np.asarray(inputs["bias"], dtype=np.float32)
    ntok = x.shape[0] // N_CORES
    blocks_by_r, ki_of, dup_cols, pairs = _plan(
        np.asarray(inputs["row_idx"]), np.asarray(inputs["col_idx"]))
    units, ops, wslot, n_wgroups, x_order, makespan, swaps = _schedule(
        blocks_by_r, ki_of, dup_cols, pairs, ntok)
    xT = np.ascontiguousarray(
        x[core * ntok:(core + 1) * ntok].T).astype(BF16).astype(np.float32)
    wT = np.transpose(weight, (0, 2, 1)).astype(BF16).astype(np.float32)
    n_th = ntok // 512
    psum = {}
    osb = {p: np.zeros((128, ntok), dtype=np.float16)
           for p in range(len(pairs))}
    yt = np.zeros((D_OUT, ntok), dtype=np.float16)
    for op in ops:
        if op[0] == "alloc":
            psum[op[1]] = [np.zeros((128, 512), dtype=np.float32)
                           for _ in range(op[2])]
        elif op[0] == "mm":
            _, ui, b, start, stop, tix = op
            ki, mi, c, w = b
            p, th = units[ui]
            acc = wT[w].T @ xT[c * 64:(c + 1) * 64,
                               th * 512:(th + 1) * 512]
            reg = psum[ui][tix][mi * 64:(mi + 1) * 64, :]
            if start:
                reg[:] = acc
            else:
                reg += acc
        elif op[0] == "evict":
            _, ui, regions, final = op
            p, th = units[ui]
            pt = psum.pop(ui)
            r1, r2 = pairs[p]
            for (mi, tix, first) in regions:
                r = (r1, r2)[mi]
                src = pt[tix][mi * 64:(mi + 1) * 64, :]
                dst = osb[p][mi * 64:(mi + 1) * 64,
                             th * 512:(th + 1) * 512]
                if first:
                    dst[:] = (src + bias[r * 64:(r + 1) * 64,
                                         None]).astype(np.float16)
                else:
                    dst[:] = (dst.astype(np.float32) + src
                              ).astype(np.float16)
        elif op[0] == "store":
            p = op[1]
            r1, r2 = pairs[p]
            for mi, r in enumerate((r1, r2)):
                if len(blocks_by_r[r]) == 0:
                    osb[p][mi * 64:(mi + 1) * 64, :] = bias[
                        r * 64:(r + 1) * 64, None].astype(np.float16)
            yt[128 * p:128 * (p + 1), :] = osb[p]
    perm = np.empty(D_OUT, dtype=np.int64)
    for p, (r1, r2) in enumerate(pairs):
        perm[r1 * BLK:(r1 + 1) * BLK] = np.arange(128 * p, 128 * p + 64)
        perm[r2 * BLK:(r2 + 1) * BLK] = np.arange(128 * p + 64,
                                                  128 * p + 128)
    y = yt[perm, :].T.astype(np.float32)
    return y, makespan, swaps, len(ops)


# -------------------------------------------------------------------- run

def kernel(x, weight, bias, row_idx, col_idx):
    x = np.asarray(x, dtype=np.float32)
    weight = np.asarray(weight, dtype=np.float32)
    bias = np.asarray(bias, dtype=np.float32)
    row_idx = np.asarray(row_idx)
    col_idx = np.asarray(col_idx)
    ntok_total = x.shape[0]
    assert ntok_total % N_CORES == 0
    ntok = ntok_total // N_CORES
    assert ntok % 512 == 0

    blocks_by_r, ki_of, dup_cols, pairs = _plan(row_idx, col_idx)
    units, ops, wslot, n_wgroups, x_order, makespan, swaps = _schedule(
        blocks_by_r, ki_of, dup_cols, pairs, ntok)
    nc = _build(blocks_by_r, ki_of, dup_cols, pairs, units, ops, wslot,
                n_wgroups, x_order, ntok)

    wg = [np.zeros((128, _wgroup_slots(g) * BLK), dtype=BF16)
          for g in range(n_wgroups)]
    wT = np.ascontiguousarray(np.transpose(weight, (0, 2, 1))).astype(BF16)
    for w, (ki, g, j) in wslot.items():
        wg[g][ki * 64:(ki + 1) * 64, j * BLK:(j + 1) * BLK] = wT[w]

    bias_pk = np.zeros((128, len(pairs)), dtype=np.float32)
    for p, (r1, r2) in enumerate(pairs):
        bias_pk[:64, p] = bias[r1 * BLK:(r1 + 1) * BLK]
        bias_pk[64:, p] = bias[r2 * BLK:(r2 + 1) * BLK]

    in_maps = []
    for cid in range(N_CORES):
        xt = np.ascontiguousarray(
            x[cid * ntok:(cid + 1) * ntok].T).astype(BF16)
        m = {"xt": xt, "bias_pk": bias_pk}
        for g in range(n_wgroups):
            m[f"wg{g}"] = wg[g]
        in_maps.append(m)

    res = run_bass_kernel_spmd(nc, in_maps, core_ids=list(range(N_CORES)))

    perm = np.empty(D_OUT, dtype=np.int64)
    for p, (r1, r2) in enumerate(pairs):
        perm[r1 * BLK:(r1 + 1) * BLK] = np.arange(128 * p, 128 * p + 64)
        perm[r2 * BLK:(r2 + 1) * BLK] = np.arange(128 * p + 64,
                                                  128 * p + 128)
    y = np.empty((ntok_total, D_OUT), dtype=np.float32)
    for cid in range(N_CORES):
        yt = res.results[cid]["yt"]
        y[cid * ntok:(cid + 1) * ntok] = yt[perm, :].T.astype(np.float32)
    return y


# revision 39
# speedup vs baseline: 1.0813x; 1.0813x over previous
"""Block-sparse linear y = x @ W^T + b on 8 TRN2 NeuronCores.

Problem shape (hardcoded): x [8192, 4096] f32, weight [1024, 64, 64] f32
(64x64 blocks), bias [4096] f32, row_idx/col_idx [1024] int32 over a 64x64
block grid.

Strategy: data-parallel over tokens (1024/core). Each core computes
y^T[feat, tok] = W x^T + b via K=64/M=64 block matmuls packed 4-wide into
the PE-array quadrants with tile_position; compute in bf16 (fp32 PSUM
accumulate), x^T resident in SBUF (two partition-phase copies so either
array row-group can serve any col-block), weights streamed as grouped DMA
transfers, bias added on evacuation via ScalarE, output stored as y^T f32
and transposed/concatenated on host.

TRN2 PSUM rule (measured): concurrent matmuls that share an output
col-group must write different PSUM banks -> row-group ki writes psum
tile[ki]; evacuation computes A + B + bias.
"""

from contextlib import ExitStack
from dataclasses import dataclass, field

import numpy as np
import ml_dtypes

import concourse.tile as tile
from concourse import bacc, mybir
from concourse.bass_utils import run_bass_kernel_spmd

BLK = 64
OUT_BLK = 64
IN_BLK = 64
D_IN = IN_BLK * BLK    # 4096
D_OUT = OUT_BLK * BLK  # 4096
N_CORES = 8
WGRP = 16              # weight tiles per DMA group
XCH = 2                # x tiles (128-row groups) per DMA chunk
BF16 = ml_dtypes.bfloat16


@dataclass
class _WTile:
    ki0: tuple | None = None   # (mi, c, w_idx) served by array rows 0-63
    ki1: tuple | None = None   # (mi, c, w_idx) served by array rows 64-127


@dataclass
class _Pair:
    r: tuple
    wtiles: list = field(default_factory=list)
    adjacent: bool = False


def _make_schedule(row_idx, col_idx):
    # keep-last dedupe of (r, c), matching jax .at[].set semantics
    d = {}
    for i in range(len(row_idx)):
        d[(int(row_idx[i]), int(col_idx[i]))] = i
    blocks_by_r = [[] for _ in range(OUT_BLK)]
    for (r, c), w in d.items():
        blocks_by_r[r].append((c, w))
    for lst in blocks_by_r:
        lst.sort()
    pairs = []
    for p in range(OUT_BLK // 2):
        r1, r2 = 2 * p, 2 * p + 1
        ps = _Pair(r=(r1, r2), adjacent=True)
        q = [
            [(0, c, w) for (c, w) in blocks_by_r[r1]],
            [(1, c, w) for (c, w) in blocks_by_r[r2]],
        ]
        t = 0
        while q[0] or q[1]:
            first = t % 2
            a = q[first].pop() if q[first] else (
                q[1 - first].pop() if q[1 - first] else None)
            b = q[1 - first].pop() if q[1 - first] else (
                q[first].pop() if q[first] else None)
            ps.wtiles.append(_WTile(ki0=a, ki1=b))
            t += 1
        pairs.append(ps)
    n_wtiles = sum(len(ps.wtiles) for ps in pairs)
    return pairs, n_wtiles


def _pack_host_arrays(weight, bias, pairs):
    n_wtiles = sum(len(ps.wtiles) for ps in pairs)
    n_groups = (n_wtiles + WGRP - 1) // WGRP
    wgrp = np.zeros((max(n_groups, 1), 128, WGRP * BLK), dtype=BF16)
    bias_pk = np.zeros((128, len(pairs)), dtype=np.float32)
    wT = np.ascontiguousarray(
        np.transpose(np.asarray(weight), (0, 2, 1))).astype(BF16)
    t = 0
    for p, ps in enumerate(pairs):
        r1, r2 = ps.r
        bias_pk[:64, p] = bias[r1 * BLK:(r1 + 1) * BLK]
        bias_pk[64:, p] = bias[r2 * BLK:(r2 + 1) * BLK]
        for wt in ps.wtiles:
            g, j = divmod(t, WGRP)
            for ki, half in ((0, wt.ki0), (1, wt.ki1)):
                if half is not None:
                    wgrp[g, ki * 64:(ki + 1) * 64,
                         j * BLK:(j + 1) * BLK] = wT[half[2]]
            t += 1
    return wgrp, bias_pk


def _x_tile_of(c, ki):
    """(copy, tile index) of the resident x^T tile serving block c on array
    row-group ki. Copy 'a' holds blocks (2b, 2b+1) on partition halves
    (0, 1); copy 'b' is shifted 64 rows: (2b+1, 2b+2), with tile 31
    wrapping to block 0."""
    if (c % 2) == ki:
        return ("a", c // 2)
    if c % 2 == 1:
        return ("b", (c - 1) // 2)
    return ("b", (c // 2 - 1) % (IN_BLK // 2))


def _build_kernel(pairs, n_wtiles, ntok, w_bufs=10, ps_bufs=8, out_bufs=6):
    assert ntok % 512 == 0
    n_th = ntok // 512
    sdt = mybir.dt.bfloat16
    f32 = mybir.dt.float32

    nc = bacc.Bacc("TRN2", target_bir_lowering=False, debug=False)
    xt_rows = D_IN + 64
    n_groups = (n_wtiles + WGRP - 1) // WGRP
    xt_d = nc.dram_tensor("xt", [xt_rows, ntok], sdt,
                          kind="ExternalInput").ap()
    wg_d = nc.dram_tensor("wgrp", [max(n_groups, 1), 128, WGRP * BLK], sdt,
                          kind="ExternalInput").ap()
    bias_d = nc.dram_tensor("bias_pk", [128, len(pairs)], f32,
                            kind="ExternalInput").ap()
    yt_d = nc.dram_tensor("yt", [D_OUT, ntok], f32,
                          kind="ExternalOutput").ap()

    with tile.TileContext(nc) as tc:
        with ExitStack() as ctx:
            xpool = ctx.enter_context(tc.tile_pool(name="xp", bufs=1))
            wpool = ctx.enter_context(tc.tile_pool(name="wp", bufs=w_bufs))
            pspool = ctx.enter_context(
                tc.tile_pool(name="ps", bufs=ps_bufs, space="PSUM"))
            opool = ctx.enter_context(tc.tile_pool(name="op", bufs=out_bufs))
            bpool = ctx.enter_context(tc.tile_pool(name="bp", bufs=1))

            bias_sb = bpool.tile([128, len(pairs)], f32, tag="bias",
                                 name="bias_sb")
            nc.sync.dma_start(bias_sb[:], bias_d[:])

            xchunks = {}

            def x_ap(c, ki, th):
                cp, b = _x_tile_of(c, ki)
                cb, wi = divmod(b, XCH)
                key = (cp, cb)
                if key not in xchunks:
                    t = xpool.tile([128, XCH * ntok], sdt, tag=f"x{cp}{cb}",
                                   name=f"x{cp}{cb}")
                    off = cb * 128 * XCH + (64 if cp == "b" else 0)
                    src = xt_d[off:off + 128 * XCH, :].rearrange(
                        "(c p) t -> p c t", p=128)
                    dst = t[:].rearrange("p (c t) -> p c t", c=XCH)
                    nc.sync.dma_start(dst, src)
                    xchunks[key] = t
                t = xchunks[key]
                o = wi * ntok + th * 512
                return t[ki * 64:(ki + 1) * 64, o:o + 512]

            nmm = [{(ki, mi): 0 for ki in (0, 1) for mi in (0, 1)}
                   for _ in pairs]
            for p, ps_ in enumerate(pairs):
                for wt in ps_.wtiles:
                    for ki, half in ((0, wt.ki0), (1, wt.ki1)):
                        if half is not None:
                            nmm[p][(ki, half[0])] += 1
            done = [{(th, ki, mi): 0 for th in range(n_th)
                     for ki in (0, 1) for mi in (0, 1)} for _ in pairs]

            psum = {}
            wg_tiles = {}

            def ensure_psum(p, th):
                if (p, th) not in psum:
                    psum[(p, th)] = [
                        pspool.tile([128, 512], f32, tag="ps",
                                    name=f"ps{p}_{th}_{k}") for k in range(2)]

            def store_out(p, th, osb):
                ps_ = pairs[p]
                ts = slice(th * 512, (th + 1) * 512)
                r1, r2 = ps_.r
                if ps_.adjacent:
                    nc.gpsimd.dma_start(yt_d[r1 * BLK:r1 * BLK + 128, ts],
                                        osb[:])
                else:
                    nc.gpsimd.dma_start(yt_d[r1 * BLK:(r1 + 1) * BLK, ts],
                                        osb[0:64, :])
                    nc.gpsimd.dma_start(yt_d[r2 * BLK:(r2 + 1) * BLK, ts],
                                        osb[64:128, :])

            def eviction_th(p, th):
                osb = opool.tile([128, 512], f32, tag="o32",
                                 name=f"o{p}_{th}")
                pt = psum.pop((p, th))
                if all(v > 0 for v in nmm[p].values()):
                    nc.scalar.activation(
                        osb[:], pt[0][:],
                        mybir.ActivationFunctionType.Identity,
                        bias=bias_sb[:, p:p + 1], scale=1.0)
                    nc.vector.tensor_add(osb[:], osb[:], pt[1][:])
                else:
                    for mi in (0, 1):
                        oh = osb[mi * 64:(mi + 1) * 64, :]
                        bh = bias_sb[mi * 64:(mi + 1) * 64, p:p + 1]
                        srcs = [pt[ki][mi * 64:(mi + 1) * 64, :]
                                for ki in (0, 1) if nmm[p][(ki, mi)] > 0]
                        if not srcs:
                            nc.vector.memset(oh, 0.0)
                            nc.vector.tensor_scalar_add(oh, oh, bh)
                        else:
                            nc.scalar.activation(
                                oh, srcs[0],
                                mybir.ActivationFunctionType.Identity,
                                bias=bh, scale=1.0)
                            if len(srcs) > 1:
                                nc.vector.tensor_add(oh, oh, srcs[1])
                store_out(p, th, osb)

            pair_base = []
            acc = 0
            for ps_ in pairs:
                pair_base.append(acc)
                acc += len(ps_.wtiles)

            for p, ps_ in enumerate(pairs):
                if not ps_.wtiles:
                    continue
                for th in range(n_th):
                    ensure_psum(p, th)
                for wt_j, wt in enumerate(ps_.wtiles):
                    idx = pair_base[p] + wt_j
                    gi, jj = divmod(idx, WGRP)
                    for gpf in (gi, gi + 1, gi + 2):
                        if gpf < n_groups and gpf not in wg_tiles:
                            wg_tiles[gpf] = wpool.tile(
                                [128, WGRP * BLK], sdt, tag="wg",
                                name=f"wg{gpf}")
                            nc.sync.dma_start(wg_tiles[gpf][:],
                                              wg_d[gpf, :, :])
                    for ki, half in ((0, wt.ki0), (1, wt.ki1)):
                        if half is None:
                            continue
                        mi, c, w = half
                        lhsT = wg_tiles[gi][ki * 64:(ki + 1) * 64,
                                            jj * BLK:(jj + 1) * BLK]
                        for th in range(n_th):
                            done[p][(th, ki, mi)] += 1
                            first = done[p][(th, ki, mi)] == 1
                            last = done[p][(th, ki, mi)] == nmm[p][(ki, mi)]
                            nc.tensor.matmul(
                                psum[(p, th)][ki][mi * 64:(mi + 1) * 64, :],
                                lhsT, x_ap(c, ki, th),
                                start=first, stop=last,
                                tile_position=(ki * 64, mi * 64),
                                skip_group_check=True,
                            )
                for th in range(n_th):
                    eviction_th(p, th)

            for p, ps_ in enumerate(pairs):
                if ps_.wtiles:
                    continue
                for th in range(n_th):
                    osb = opool.tile([128, 512], f32, tag="o32",
                                     name=f"oz{p}_{th}")
                    nc.vector.memset(osb[:], 0.0)
                    nc.vector.tensor_scalar_add(osb[:], osb[:],
                                                bias_sb[:, p:p + 1])
                    store_out(p, th, osb)
    nc.compile()
    return nc


def kernel(x, weight, bias, row_idx, col_idx):
    x = np.asarray(x, dtype=np.float32)
    weight = np.asarray(weight, dtype=np.float32)
    bias = np.asarray(bias, dtype=np.float32)
    row_idx = np.asarray(row_idx)
    col_idx = np.asarray(col_idx)
    ntok_total = x.shape[0]
    assert ntok_total % N_CORES == 0
    ntok = ntok_total // N_CORES

    pairs, n_wt = _make_schedule(row_idx, col_idx)
    wgrp, bias_pk = _pack_host_arrays(weight, bias, pairs)
    nc = _build_kernel(pairs, n_wt, ntok)

    in_maps = []
    for c in range(N_CORES):
        xt = np.ascontiguousarray(
            x[c * ntok:(c + 1) * ntok].T).astype(BF16)
        xt = np.concatenate([xt, xt[:64]], axis=0)
        in_maps.append({"xt": xt, "wgrp": wgrp, "bias_pk": bias_pk})

    res = run_bass_kernel_spmd(nc, in_maps, core_ids=list(range(N_CORES)))
    y = np.empty((ntok_total, D_OUT), dtype=np.float32)
    for c in range(N_CORES):
        y[c * ntok:(c + 1) * ntok] = res.results[c]["yt"].T
    return y

